# revision 4
# baseline (speedup 1.0000x reference)
"""Trainium2 Bass kernel for nn_CrossAttention_28767690949003.

Full (unsharded) inputs in, full output out.  Internally shards across 8
NeuronCores as (batch b in {0,1}) x (head-group g in {0..3}, 2 heads each):
Wq/Wk/Wv column-sharded on inner_dim, Wo row-sharded; the all-reduce after
to_out is realised as a host-side partial sum over the 4 head-group cores of
each batch.

Per-core kernel (Tile framework), all sizes hardcoded:
  x_b [4096,512], ctx_b [4096,768] -> PE-transpose 128x128 blocks ->
  Q^T [128,4096], K^T [128,4096] (fp32r matmuls), V' [k,65]-per-head with a
  ones column (gives softmax denominators for free in the attn@V matmul).
  Attention runs per 512-query block: S^T = K @ Q^T (two heads row-packed on
  the PE), exp on the scalar engine over [128,1024] psum spans (scale=1/8
  folded into the activation), O~^T accumulated over 32 k-chunks in psum.
  Output projection uses O~^T directly as lhsT with a ones-column in the
  padded Wo to produce the transposed softmax denominators Z^T [q,1]; the
  per-head 1/Z is applied on the vector engine after the projection
  (legal because the projection is linear per head).
"""

import numpy as np

import concourse.bass as bass
import concourse.mybir as mybir
import concourse.tile as tile
from concourse import bacc
from concourse.bass import ds, ts
from concourse.bass_utils import run_bass_kernel_spmd
from concourse.masks import make_identity

B, QL, KL = 2, 4096, 4096
QD, CD = 512, 768
H, DH = 8, 64
SCALE = DH**-0.5
NCORES, GROUPS, DG = 8, 4, 128  # DG inner dims per head-group (2 heads)

P = 128
FP32 = mybir.dt.float32
FP32R = mybir.dt.float32r
AF = mybir.ActivationFunctionType

USE_FP32R = True
MM_DT = FP32R if USE_FP32R else FP32


def build_cross_attn(tc, outs, ins):
    nc = tc.nc
    x, cx, wq, wk, wv, wo = ins
    (y,) = outs

    with (
        tc.tile_pool(name="const", bufs=1) as const,
        tc.tile_pool(name="persist", bufs=1) as persist,
    ):
        ident = const.tile([P, P], FP32, name="ident")
        make_identity(nc, ident)
        wq_sb = const.tile([P, 4, DG], MM_DT, name="wq_sb")
        nc.sync.dma_start(wq_sb, wq.rearrange("(o p) d -> p o d", p=P))
        wk_sb = const.tile([P, 6, DG], MM_DT, name="wk_sb")
        nc.sync.dma_start(wk_sb, wk.rearrange("(o p) d -> p o d", p=P))
        wv_sb = const.tile([P, 6, DG], MM_DT, name="wv_sb")
        nc.sync.dma_start(wv_sb, wv.rearrange("(o p) d -> p o d", p=P))
        # wo comes pre-packed from the host as [65, 2, 514]:
        #   [0:64, h, 0:512] = Wo rows of head h, [64, h, 512:514] = 1.0
        # (two ones columns: fp32r matmuls need moving free >= 2)
        wo_sb = const.tile([65, 2, 514], MM_DT, name="wo_sb")
        nc.sync.dma_start(wo_sb, wo)

        qt_sb = persist.tile([P, 8, 512], MM_DT, name="qt_sb")  # Q^T [dg, QL]
        kt_sb = persist.tile([P, 8, 512], MM_DT, name="kt_sb")  # K^T [dg, KL]
        # V' per k-chunk: cols 0:64 head0 V, 64 ones, 65:129 head1 V, 129 ones
        v_sb = persist.tile([P, 32, 130], MM_DT, name="v_sb")
        ones_c = const.tile([P, 1], FP32, name="ones_c")
        nc.gpsimd.memset(ones_c, 1.0)
        ones_b = ones_c[:, None, :].to_broadcast((P, 32, 1))
        nc.vector.tensor_copy(v_sb[:, :, 64:65], ones_b)
        nc.vector.tensor_copy(v_sb[:, :, 129:130], ones_b)

        # ---- Phase A: transposes + Q/K/V projections ----
        with (
            tc.tile_pool(name="aio", bufs=2) as aio,
            tc.tile_pool(name="atr", bufs=2) as atr,
            tc.tile_pool(name="aps", bufs=2, space="PSUM") as aps,
        ):
            for g8 in range(8):  # 512-row groups of both QL and KL
                ctx_t = aio.tile([P, 4, CD], FP32, name="ctx_t")
                nc.sync.dma_start(
                    ctx_t, cx[ds(g8 * 512, 512), :].rearrange("(a p) d -> p a d", p=P)
                )
                ctxT = atr.tile([P, 6, 512], MM_DT, name="ctxT")
                for cc in range(6):
                    tp = aps.tile([P, 512], FP32, name="tp", tag="tp")
                    for a in range(4):
                        nc.tensor.transpose(
                            tp[:, ts(a, P)], ctx_t[:, a, ts(cc, P)], ident
                        )
                    if cc % 2 == 0:
                        nc.vector.tensor_copy(ctxT[:, cc, :], tp)
                    else:
                        nc.scalar.copy(ctxT[:, cc, :], tp)

                kp = aps.tile([P, 512], FP32, name="kp", tag="pj")
                for cc in range(6):
                    nc.tensor.matmul(
                        kp,
                        wk_sb[:, cc, :],
                        ctxT[:, cc, :],
                        start=(cc == 0),
                        stop=(cc == 5),
                    )
                nc.scalar.copy(kt_sb[:, g8, :], kp)

                for a in range(4):
                    vp = aps.tile([P, DG], FP32, name="vp", tag="vp")
                    for cc in range(6):
                        nc.tensor.matmul(
                            vp,
                            ctxT[:, cc, ts(a, P)],
                            wv_sb[:, cc, :],
                            start=(cc == 0),
                            stop=(cc == 5),
                        )
                    kt = g8 * 4 + a
                    nc.vector.tensor_copy(v_sb[:, kt, 0:64], vp[:, 0:64])
                    nc.vector.tensor_copy(v_sb[:, kt, 65:129], vp[:, 64:128])

                x_t = aio.tile([P, 4, QD], FP32, name="x_t")
                nc.sync.dma_start(
                    x_t, x[ds(g8 * 512, 512), :].rearrange("(a p) d -> p a d", p=P)
                )
                xT = atr.tile([P, 4, 512], MM_DT, name="xT")
                for cc in range(4):
                    tp = aps.tile([P, 512], FP32, name="tp", tag="tp")
                    for a in range(4):
                        nc.tensor.transpose(
                            tp[:, ts(a, P)], x_t[:, a, ts(cc, P)], ident
                        )
                    if cc % 2 == 0:
                        nc.vector.tensor_copy(xT[:, cc, :], tp)
                    else:
                        nc.scalar.copy(xT[:, cc, :], tp)

                qp = aps.tile([P, 512], FP32, name="qp", tag="pj")
                for cc in range(4):
                    nc.tensor.matmul(
                        qp,
                        wq_sb[:, cc, :],
                        xT[:, cc, :],
                        start=(cc == 0),
                        stop=(cc == 3),
                    )
                nc.scalar.copy(qt_sb[:, g8, :], qp)

        # ---- Phase B: attention + output projection ----
        with (
            tc.tile_pool(name="bS", bufs=2, space="PSUM") as psS,
            tc.tile_pool(name="bO", bufs=4, space="PSUM") as psO,
            tc.tile_pool(name="bE", bufs=3) as eP,
            tc.tile_pool(name="bT", bufs=2) as tP,
            tc.tile_pool(name="bOut", bufs=3) as oP,
        ):
            for qb in range(8):  # 512-query blocks
                o0 = psO.tile([65, 512], FP32, name="o0", tag="oyz")
                o1 = psO.tile([65, 512], FP32, name="o1", tag="oyz")
                for kc in range(32):  # 128-key chunks
                    sp = psS.tile([P, 1024], FP32, name="sp", tag="sp")
                    g8, off = kc // 4, (kc % 4) * P
                    nc.tensor.matmul(
                        sp[:, 0:512],
                        kt_sb[0:64, g8, ds(off, P)],
                        qt_sb[0:64, qb, :],
                        start=True,
                        stop=True,
                    )
                    nc.tensor.matmul(
                        sp[:, 512:1024],
                        kt_sb[64:P, g8, ds(off, P)],
                        qt_sb[64:P, qb, :],
                        start=True,
                        stop=True,
                    )
                    ee = eP.tile([P, 1024], MM_DT, name="ee")
                    nc.scalar.activation(ee, sp, AF.Exp, scale=SCALE)
                    nc.tensor.matmul(
                        o0,
                        v_sb[:, kc, 0:65],
                        ee[:, 0:512],
                        start=(kc == 0),
                        stop=(kc == 31),
                    )
                    nc.tensor.matmul(
                        o1,
                        v_sb[:, kc, 65:130],
                        ee[:, 512:1024],
                        start=(kc == 0),
                        stop=(kc == 31),
                    )

                ot0 = tP.tile([65, 512], MM_DT, name="ot0")
                ot1 = tP.tile([65, 512], MM_DT, name="ot1")
                nc.vector.tensor_copy(ot0, o0)
                nc.vector.tensor_copy(ot1, o1)
                zp = psO.tile([P, 16], FP32, name="zp", tag="oyz")
                rz = tP.tile([P, 16], FP32, name="rz")
                for qc in range(4):
                    y0 = psO.tile([P, 512], FP32, name="y0", tag="oyz")
                    y1 = psO.tile([P, 512], FP32, name="y1", tag="oyz")
                    nc.tensor.matmul(
                        y0,
                        ot0[:, ts(qc, P)],
                        wo_sb[:, 0, 0:512],
                        start=True,
                        stop=True,
                    )
                    nc.tensor.matmul(
                        zp[:, ds(qc * 4, 2)],
                        ot0[:, ts(qc, P)],
                        wo_sb[:, 0, 512:514],
                        start=True,
                        stop=True,
                    )
                    nc.tensor.matmul(
                        y1,
                        ot1[:, ts(qc, P)],
                        wo_sb[:, 1, 0:512],
                        start=True,
                        stop=True,
                    )
                    nc.tensor.matmul(
                        zp[:, ds(qc * 4 + 2, 2)],
                        ot1[:, ts(qc, P)],
                        wo_sb[:, 1, 512:514],
                        start=True,
                        stop=True,
                    )
                    nc.vector.reciprocal(rz[:, ds(qc * 4, 4)], zp[:, ds(qc * 4, 4)])
                    t0 = oP.tile([P, 512], FP32, name="t0")
                    nc.vector.tensor_scalar_mul(t0, y0, rz[:, ds(qc * 4, 1)])
                    yo = oP.tile([P, 512], FP32, name="yo")
                    nc.vector.tensor_scalar_mul(yo, y1, rz[:, ds(qc * 4 + 2, 1)])
                    nc.vector.tensor_add(yo, yo, t0)
                    nc.sync.dma_start(y[ds(qb * 512 + qc * P, P), :], yo)


_CACHE = {}


def _get_nc():
    if "nc" not in _CACHE:
        nc = bacc.Bacc(
            "TRN2", target_bir_lowering=False, debug=False, enable_asserts=False
        )
        x = nc.dram_tensor("x", [QL, QD], FP32, kind="ExternalInput")
        cx = nc.dram_tensor("cx", [KL, CD], FP32, kind="ExternalInput")
        wq = nc.dram_tensor("wq", [QD, DG], MM_DT, kind="ExternalInput")
        wk = nc.dram_tensor("wk", [CD, DG], MM_DT, kind="ExternalInput")
        wv = nc.dram_tensor("wv", [CD, DG], MM_DT, kind="ExternalInput")
        wo = nc.dram_tensor("wo", [65, 2, 514], MM_DT, kind="ExternalInput")
        y = nc.dram_tensor("y", [QL, QD], FP32, kind="ExternalOutput")
        with tile.TileContext(nc) as tc:
            build_cross_attn(
                tc, [y.ap()], [t.ap() for t in (x, cx, wq, wk, wv, wo)]
            )
        nc.compile()
        _CACHE["nc"] = nc
    return _CACHE["nc"]


def make_in_maps(x, context, Wq, Wk, Wv, Wo):
    x = np.asarray(x, np.float32)
    context = np.asarray(context, np.float32)
    Wq, Wk, Wv, Wo = (np.asarray(a, np.float32) for a in (Wq, Wk, Wv, Wo))
    in_maps = []
    for core in range(NCORES):
        b, g = core // GROUPS, core % GROUPS
        sl = slice(g * DG, (g + 1) * DG)
        wo_p = np.zeros((65, 2, 514), np.float32)
        for h in range(2):
            rows = slice(g * DG + h * DH, g * DG + (h + 1) * DH)
            wo_p[0:64, h, 0:512] = Wo[rows, :]
            wo_p[64, h, 512:514] = 1.0
        in_maps.append(
            {
                "x": np.ascontiguousarray(x[b]),
                "cx": np.ascontiguousarray(context[b]),
                "wq": np.ascontiguousarray(Wq[:, sl]),
                "wk": np.ascontiguousarray(Wk[:, sl]),
                "wv": np.ascontiguousarray(Wv[:, sl]),
                "wo": wo_p,
            }
        )
    return in_maps


def kernel(x, context, Wq, Wk, Wv, Wo, bo):
    nc = _get_nc()
    in_maps = make_in_maps(x, context, Wq, Wk, Wv, Wo)
    res = run_bass_kernel_spmd(nc, in_maps, core_ids=list(range(NCORES)))
    out = np.zeros((B, QL, QD), np.float32)
    for core in range(NCORES):
        out[core // GROUPS] += res.results[core]["y"]
    out += np.asarray(bo, np.float32)[None, None, :]
    return out
